# revision 46
# baseline (speedup 1.0000x reference)
"""Trainium2 Bass kernel for nn_BilateralFilter (exact Gaussian bilateral filter).

Math (per reference):
  feats f_i in R^6 (scaled spatial zyx + scaled rgb), N = 12*24*24 = 6912
  sq[i,j] = |f_i - f_j|^2 ;  K = exp(-0.5*sq)
  out[c,j] = (sum_i q[c,i] K[i,j]) / (sum_i K[i,j] + eps)

Device strategy (8 cores, column-sharded over the N x N kernel):
  Each core owns J = N/8 = 864 output columns j and walks 54 i-tiles of
  128 rows, grouped in 18 triples.  Per triple:
    - mm1 x3: full-array matmuls (aug-feature contraction, bf16 two-level
      hi/lo split stack [Ah;Ah;Al].[Bh;Bl;Bh] = 24 real rows zero-padded
      to K=128) produce arg = -0.5*|f_i - f_j|^2 into 2-bank PSUM tiles.
      Full-array (NumWeights==128) matmuls are what keep the PE HAM
      activity monitor at 2.4 GHz; row-tiled K=24 variants are 2x faster
      on paper but the clock gate re-throttles to 1.2 GHz (measured,
      selectable via BILATERAL_STYLE=rowtile).
    - exp: tiles alternate between two engines working in parallel (exp
      is the critical path; ScalarE alone runs 1 elem/lane/cycle @ 1.2
      GHz = ~53us for all 54 tiles):
        ScalarE (28 tiles): exact exp via ACTIVATE (fp32 PSUM -> fp16);
        VectorE (26 tiles): Schraudolph exp: ONE tensor_scalar computes
          round(arg*1477.32 + 15315.25) saturated to uint16 (negatives
          clamp to 0 = fp16 +0.0) whose bits ARE fp16 exp(arg) to ~3%.
          The +-3% per-entry error cancels between the filtered
          numerator and the norm denominator, leaving ~1e-3 in the out.
    - mm2 x3, COL-TILED and software-pipelined one triple behind the
      exp: strip s holds [q0,q1,ones] weights in array col-group 32s,
      contracting the fp16 K-tile into a 3-row accumulator slice
      acc[32s:32s+3]; with all three K-tiles ready the triple issues
      back-to-back and the matmuls overlap in the array (~180ns/tile).
  A short burst of full-array matmuls on random data at t=0 (during the
  input DMAs) trips the HAM clock-gate before the real work begins.
  The N x N kernel matrix never touches HBM.  Host sums the 3 strip
  partials, adds eps, divides.
"""

import os
import numpy as np

try:
    import concourse.bass as bass  # noqa: F401
except ImportError:  # fresh grading dir: repo not on sys.path
    import sys

    sys.path.insert(0, "/opt/trn_rl_repo")
    import concourse.bass as bass  # noqa: F401

import concourse.mybir as mybir
import concourse.tile as tile
from concourse import bacc
from concourse.bass_utils import run_bass_kernel_spmd
SIGMA_ALPHA = (5.0, 5.0, 5.0)
SIGMA_BETA = 0.3
EPS = float(np.finfo("float").eps)

D, H, W = 12, 24, 24
N = D * H * W  # 6912
M_CORES = 8
J = N // M_CORES  # 864 output columns per core
NT = N // 128  # 54 i-tiles
NG = NT // 3  # 18 row-tiled triples
KA = 24  # stacked aug-feature contraction rows
N_WARM = 3  # bridge PE-busy until the real full-array mm1s flow

# Schraudolph fp16-bits exp: bits = round(g*SCH_A + SCH_B) clipped to u16.
# Calibrated for the measured HW semantics (round-to-nearest, saturating
# conversion): max rel err 3.02% over g in [-30, 0], mean-centered.
SCH_A = 1477.3196
SCH_B = 15315.25

_BUILD_CACHE: dict[str, object] = {}
_HY: dict[str, object] = {}

# "rowtile": 3-way row-tiled mm1 triples + per-group full-array warm matmul.
# "fullarray": serial full-array mm1 (zero-padded K=128).
_STYLE = os.environ.get("BILATERAL_STYLE", "fullarray")


def _is_act_tile(t):
    # ~28 exact-exp tiles on ScalarE, ~26 Schraudolph tiles on VectorE
    # (ACT is slightly faster per tile: 981ns vs 1058ns).
    return t % 2 == 0 or t in (27, 53)


def _build_nc():
    f16 = mybir.dt.float16
    u16 = mybir.dt.uint16
    bf16 = mybir.dt.bfloat16
    f32 = mybir.dt.float32

    nc = bacc.Bacc(None, target_bir_lowering=False)

    if _STYLE == "rowtile":
        # Row-tiled: strip s (partitions 32s..32s+23) of column group G holds
        # i-tile t = 3G+s; B is replicated at the three strip offsets.
        a_dram = nc.dram_tensor("a128", [88, NG * 128], bf16, kind="ExternalInput")
        b_dram = nc.dram_tensor("b128", [88, J], bf16, kind="ExternalInput")
    elif _STYLE == "hybrid":
        # Tile 3G: full-array (zero-padded K=128, keeps HAM warm).  Tiles
        # 3G+1 / 3G+2: row-tiled K=24 pair in strips 0 / 32 (concurrent).
        a_dram = nc.dram_tensor("a128", [128, NG * 128], bf16, kind="ExternalInput")
        b_dram = nc.dram_tensor("b128", [128, J], bf16, kind="ExternalInput")
        ar_dram = nc.dram_tensor("arow", [64, NG * 128], bf16, kind="ExternalInput")
        br_dram = nc.dram_tensor("brow", [64, J], bf16, kind="ExternalInput")
    else:
        # Full-array: operands zero-padded from K=24 to K=128 (NumWeights==128
        # keeps the HAM activity monitor warm + fast weight load).  Only the
        # 24 real rows travel over DMA; rows 24-127 are zeroed on-chip.
        a_dram = nc.dram_tensor("a128", [128, N], bf16, kind="ExternalInput")
        b_dram = nc.dram_tensor("b128", [128, J], bf16, kind="ExternalInput")
    wrm_dram = nc.dram_tensor("wrm", [128, 512], bf16, kind="ExternalInput")
    # qa3 comes pre-arranged in the exact SBUF layout [128, NT*3] so the
    # DMA is one contiguous 324B-per-partition transfer (a [N,3]->SBUF
    # scatter would cost 6912 six-byte DMA descriptors).
    qa_dram = nc.dram_tensor("qa3", [128, NT * 3], f16, kind="ExternalInput")
    out_dram = nc.dram_tensor("acc_out", [9, J], f32, kind="ExternalOutput")

    with tile.TileContext(nc) as tc:
        with (
            tc.tile_pool(name="const", bufs=1) as cpool,
            tc.tile_pool(name="kpool", bufs=8) as kpool,
            tc.tile_pool(name="gpsum", bufs=3, space="PSUM") as gpool,
            tc.tile_pool(name="apsum", bufs=1, space="PSUM") as apool,
            tc.tile_pool(name="opool", bufs=1) as opool,
        ):
            if _STYLE == "rowtile":
                A24 = cpool.tile([88, NG * 128], bf16)
                B24 = cpool.tile([88, J], bf16)
            elif _STYLE == "hybrid":
                A24 = cpool.tile([128, NG * 128], bf16)
                B24 = cpool.tile([128, J], bf16)
                AR = cpool.tile([64, NG * 128], bf16)
                BR = cpool.tile([64, J], bf16)
            else:
                A24 = cpool.tile([128, N], bf16)
                B24 = cpool.tile([128, J], bf16)
            QA3 = cpool.tile([128, NT * 3], f16)

            # Dummy ACT pulls the ~2.7us exp-table load off the critical path.
            # memsets run on the otherwise-idle GpSimd engine so VectorE and
            # the DMA queue stay clear for the pipeline.
            dmy_in = cpool.tile([128, 32], f32)
            dmy_out = cpool.tile([128, 32], f16)
            nc.gpsimd.memset(dmy_in[:], -1.0)
            nc.scalar.activation(
                dmy_out[:], dmy_in[:], mybir.ActivationFunctionType.Exp
            )

            # PE warmup: full-array matmuls on RANDOM data during the DMA
            # phase trip the HAM activity monitor so the clock gate opens
            # (1.2 -> 2.4 GHz) before the real matmuls begin.  Partial-array
            # (row/col-tiled) matmuls sustain the warm state but cannot fire
            # it, and zero operands generate no switching activity at all.
            if N_WARM > 0:
                # Varying nonzero data generated on-chip (no DMA wait): the
                # HAM activity monitor ignores zero/constant operands.
                WRM = cpool.tile([128, 512], bf16)
                nc.gpsimd.iota(
                    WRM[:], [[1, 512]], base=1, channel_multiplier=3,
                    allow_small_or_imprecise_dtypes=True,
                )
                gwarm = gpool.tile([128, J], f32, tag="g")
                for _ in range(N_WARM):
                    nc.tensor.matmul(
                        gwarm[:, 0:512], WRM[:, 0:128], WRM[:], start=True, stop=True
                    )

            if _STYLE == "rowtile":
                nc.sync.dma_start(B24[:], b_dram[:])
                nc.sync.dma_start(QA3[:], qa_dram[:])
                AW = A24.shape[1]
                nc.sync.dma_start(A24[:, 0 : AW // 4], a_dram[:, 0 : AW // 4])
                nc.sync.dma_start(A24[:, AW // 4 : AW], a_dram[:, AW // 4 : AW])
            elif _STYLE == "hybrid":
                nc.sync.dma_start(B24[:], b_dram[:])
                nc.scalar.dma_start(QA3[:], qa_dram[:])
                nc.scalar.dma_start(BR[:], br_dram[:])
                nc.sync.dma_start(A24[:, 0:512], a_dram[:, 0:512])
                nc.scalar.dma_start(AR[:, 0:512], ar_dram[:, 0:512])
                nc.sync.dma_start(A24[:, 512 : NG * 128], a_dram[:, 512 : NG * 128])
                nc.scalar.dma_start(AR[:, 512 : NG * 128], ar_dram[:, 512 : NG * 128])
            else:
                # Small transfers ride the Scalar queue (idle until the first
                # real exp); the big A operand streams on the Sync queue in
                # three chunks so the first i-tiles start early.
                nc.scalar.dma_start(B24[:], b_dram[:])
                nc.scalar.dma_start(QA3[:], qa_dram[:])
                nc.sync.dma_start(A24[:, 0:768], a_dram[:, 0:768])
                nc.sync.dma_start(A24[:, 768:3072], a_dram[:, 768:3072])
                nc.sync.dma_start(A24[:, 3072:N], a_dram[:, 3072:N])

            acc = apool.tile([128, J], f32)

            out_sb = opool.tile([128, J], f32)

            def emit_mm2_group(G, ks):
                # col-tiled triple, one 3-row accumulator slice per strip; by
                # emission time all three k tiles are ready, so the three
                # strips' matmuls issue back-to-back and overlap in the array.
                last = G == NG - 1
                for s in range(3):
                    cs = slice((3 * G + s) * 3, (3 * G + s) * 3 + 3)
                    nc.tensor.matmul(
                        acc[32 * s : 32 * s + 3, 0:512], QA3[:, cs], ks[s][:, 0:512],
                        start=(G == 0), stop=last, tile_position=(0, 32 * s),
                    )
                    nc.tensor.matmul(
                        acc[32 * s : 32 * s + 3, 512:J], QA3[:, cs], ks[s][:, 512:J],
                        start=(G == 0), stop=last, tile_position=(0, 32 * s),
                    )
                if last:
                    # Drain the accumulator on both PSUM-reading engines.
                    nc.scalar.copy(out_sb[:, 0:512], acc[:, 0:512])
                    nc.vector.tensor_copy(out_sb[:, 512:J], acc[:, 512:J])

            pending = []
            for G in range(NG):
                gs = []
                if _STYLE == "rowtile":
                    # One full-array matmul on random data per group keeps the
                    # HAM activity monitor warm (row/col-tiled matmuls alone
                    # let it re-throttle).  It writes into the same g buffer
                    # the first mm1 then fully overwrites (start=True), so it
                    # costs no extra PSUM.
                    g0 = gpool.tile([128, J], f32, tag="g")
                    nc.tensor.matmul(
                        g0[:, 0:512], WRM[:, 0:128], WRM[:], start=True, stop=True
                    )
                    for s in range(3):
                        t = 3 * G + s
                        lhs = A24[32 * s : 32 * s + KA, G * 128 : (G + 1) * 128]
                        rhs = B24[32 * s : 32 * s + KA, :]
                        g = g0 if s == 0 else gpool.tile([128, J], f32, tag="g")
                        nc.tensor.matmul(
                            g[:, 0:512], lhs, rhs[:, 0:512],
                            start=True, stop=True, tile_position=(32 * s, 0),
                        )
                        nc.tensor.matmul(
                            g[:, 512:J], lhs, rhs[:, 512:J],
                            start=True, stop=True, tile_position=(32 * s, 0),
                        )
                        gs.append(g)
                elif _STYLE == "hybrid":
                    Gb = slice(G * 128, (G + 1) * 128)
                    g0 = gpool.tile([128, J], f32, tag="g")
                    nc.tensor.matmul(g0[:, 0:512], A24[:, Gb], B24[:, 0:512], start=True, stop=True)
                    nc.tensor.matmul(g0[:, 512:J], A24[:, Gb], B24[:, 512:J], start=True, stop=True)
                    g1 = gpool.tile([128, J], f32, tag="g")
                    g2 = gpool.tile([128, J], f32, tag="g")
                    for chunk in (slice(0, 512), slice(512, J)):
                        nc.tensor.matmul(
                            g1[:, chunk], AR[0:KA, Gb], BR[0:KA, chunk],
                            start=True, stop=True, tile_position=(0, 0),
                        )
                        nc.tensor.matmul(
                            g2[:, chunk], AR[32 : 32 + KA, Gb], BR[32 : 32 + KA, chunk],
                            start=True, stop=True, tile_position=(32, 0),
                        )
                    gs.extend([g0, g1, g2])
                else:
                    for s in range(3):
                        t = 3 * G + s
                        lhs = A24[:, t * 128 : (t + 1) * 128]
                        g = gpool.tile([128, J], f32, tag="g")
                        nc.tensor.matmul(g[:, 0:512], lhs, B24[:, 0:512], start=True, stop=True)
                        nc.tensor.matmul(g[:, 512:J], lhs, B24[:, 512:J], start=True, stop=True)
                        gs.append(g)

                ks = []
                for s in range(3):
                    t = 3 * G + s
                    k = kpool.tile([128, J], f16)
                    if _is_act_tile(t):
                        nc.scalar.activation(
                            k[:], gs[s][:], mybir.ActivationFunctionType.Exp
                        )
                    else:
                        nc.vector.tensor_scalar(
                            k[:].bitcast(u16), gs[s][:], SCH_A, SCH_B,
                            mybir.AluOpType.mult, mybir.AluOpType.add,
                        )
                    ks.append(k)

                pending.append((G, ks))
                if len(pending) > 1:
                    emit_mm2_group(*pending.pop(0))
            for item in pending:
                emit_mm2_group(*item)

            # One output DMA per queue so the ~0.7us instruction overheads
            # overlap instead of serializing.
            nc.sync.dma_start(out_dram[0:3, :], out_sb[0:3, :])
            nc.scalar.dma_start(out_dram[3:6, :], out_sb[32:35, :])
            nc.gpsimd.dma_start(out_dram[6:9, :], out_sb[64:67, :])

    nc.compile()
    return nc


def _get_nc():
    nc = _BUILD_CACHE.get(_STYLE)
    if nc is None:
        nc = _build_nc()
        _BUILD_CACHE[_STYLE] = nc
    return nc


def _split_bf16_2(a):
    import ml_dtypes

    bf = ml_dtypes.bfloat16
    a = np.asarray(a, dtype=np.float32)
    h = a.astype(bf)
    l = (a - h.astype(np.float32)).astype(bf)
    return h, l


def _host_prep(q_in, image, v_alpha, v_beta):
    """Augmented feature matrices (fp32, O(N) work only)."""
    import ml_dtypes

    q_in = np.asarray(q_in, dtype=np.float32)
    image = np.asarray(image, dtype=np.float32)
    v_alpha = np.asarray(v_alpha, dtype=np.float32)
    v_beta = np.asarray(v_beta, dtype=np.float32)

    z = np.arange(D, dtype=np.float32)[:, None, None]
    y = np.arange(H, dtype=np.float32)[None, :, None]
    x = np.arange(W, dtype=np.float32)[None, None, :]
    shp = (D, H, W)
    zz = np.broadcast_to(v_alpha[0] * z / np.float32(SIGMA_ALPHA[0]), shp)
    xx = np.broadcast_to(v_alpha[1] * x / np.float32(SIGMA_ALPHA[1]), shp)
    yy = np.broadcast_to(v_alpha[2] * y / np.float32(SIGMA_ALPHA[2]), shp)
    xyz = np.stack([zz, yy, xx], axis=3)
    rgb = v_beta * np.transpose(image, (1, 2, 3, 0)) / np.float32(SIGMA_BETA)
    feats = np.concatenate([xyz, rgb], axis=3).reshape(-1, 6).astype(np.float32)

    # Center each feature dim: |f_i - f_j| is translation invariant, smaller
    # magnitudes mean less cancellation / split error.
    feats = feats - (feats.min(axis=0) + feats.max(axis=0)) * np.float32(0.5)

    s = np.einsum("nf,nf->n", feats, feats).astype(np.float32)

    F = 8
    a_all = np.empty((F, N), dtype=np.float32)
    a_all[0:6] = feats.T
    a_all[6] = -0.5 * s
    a_all[7] = 1.0

    b_full = np.empty((F, N), dtype=np.float32)
    b_full[0:6] = feats.T
    b_full[6] = 1.0
    b_full[7] = -0.5 * s

    # Two-level bf16 split, exact cross terms kept: g = Ah.Bh + Ah.Bl + Al.Bh
    # (dropped Al.Bl ~ 1e-4 absolute in arg).  K = 3*F = 24 real rows.
    ah, al = _split_bf16_2(a_all)
    bh, bl = _split_bf16_2(b_full)
    bf = ml_dtypes.bfloat16
    a24 = np.concatenate([ah, ah, al], axis=0)
    b24 = np.concatenate([bh, bl, bh], axis=0)
    if _STYLE == "rowtile":
        a128 = np.zeros((88, NG * 128), dtype=bf)
        for t in range(NT):
            G, sp = divmod(t, 3)
            a128[32 * sp : 32 * sp + KA, G * 128 : (G + 1) * 128] = a24[
                :, t * 128 : (t + 1) * 128
            ]
        b128 = np.asarray(b24, dtype=bf)  # replicated per-core in _in_maps
    elif _STYLE == "hybrid":
        a128 = np.zeros((128, NG * 128), dtype=bf)
        arow = np.zeros((64, NG * 128), dtype=bf)
        for G in range(NG):
            cb = slice(G * 128, (G + 1) * 128)
            a128[0:KA, cb] = a24[:, (3 * G) * 128 : (3 * G + 1) * 128]
            arow[0:KA, cb] = a24[:, (3 * G + 1) * 128 : (3 * G + 2) * 128]
            arow[32 : 32 + KA, cb] = a24[:, (3 * G + 2) * 128 : (3 * G + 3) * 128]
        b128 = np.zeros((128, N), dtype=bf)
        b128[0:KA] = b24
        _HY["arow"] = arow
    else:
        a128 = np.zeros((128, N), dtype=bf)
        b128 = np.zeros((128, N), dtype=bf)
        a128[0:KA] = a24
        b128[0:KA] = b24

    qa3 = np.empty((N, 3), dtype=np.float16)
    qa3[:, 0] = q_in[0].reshape(-1).astype(np.float16)
    qa3[:, 1] = q_in[1].reshape(-1).astype(np.float16)
    qa3[:, 2] = np.float16(1.0)
    # Pre-arrange into the SBUF layout [128, NT*3]: partition p, slot t*3+c
    # holds qa3[t*128 + p, c].
    qa3 = np.ascontiguousarray(
        qa3.reshape(NT, 128, 3).transpose(1, 0, 2).reshape(128, NT * 3)
    )
    return a128, b128, qa3


def _in_maps(a128, b128, qa3):
    import ml_dtypes

    bf = ml_dtypes.bfloat16
    wrm = (
        np.random.default_rng(1234)
        .uniform(-1.0, 1.0, (128, 512))
        .astype(np.float32)
        .astype(bf)
    )
    maps = []
    for c in range(M_CORES):
        slab = np.ascontiguousarray(b128[:, c * J : (c + 1) * J])
        if _STYLE == "rowtile":
            b = np.zeros((88, J), dtype=bf)
            for sp in range(3):
                b[32 * sp : 32 * sp + KA, :] = slab
        else:
            b = slab
        m = {"a128": a128, "b128": b, "qa3": qa3, "wrm": wrm}
        if _STYLE == "hybrid":
            brow = np.zeros((64, J), dtype=bf)
            brow[0:KA] = slab[0:KA]
            brow[32 : 32 + KA] = slab[0:KA]
            m["arow"] = _HY["arow"]
            m["brow"] = brow
        maps.append(m)
    return maps


def kernel(q_in, image, v_alpha, v_beta):
    a128, b128, qa3 = _host_prep(q_in, image, v_alpha, v_beta)

    nc = _get_nc()
    res = run_bass_kernel_spmd(
        nc, _in_maps(a128, b128, qa3), core_ids=list(range(M_CORES))
    )

    acc = np.concatenate(
        [res.results[c]["acc_out"] for c in range(M_CORES)], axis=1
    )  # [9, N]
    a3 = acc.reshape(3, 3, N)
    filtered = a3[:, 0:2, :].sum(axis=0)
    norm = a3[:, 2, :].sum(axis=0)
    out = filtered / (norm[None, :] + EPS)
    return out.reshape(2, D, H, W).astype(np.float32)
